# revision 33
# baseline (speedup 1.0000x reference)
"""Trainium2 Bass kernel for nn_Attention_33054068310137 (8-core SPMD).

Math: the reference computes, per (head h, batch b):
    blend = (1-w_h) * cosine + w_h * cov        # [N, N]
    out_h = blend @ fv                          # [N, DH]
with cosine[n,m] = (fq[n].fk[m])/(|fq[n]||fk[m]|) and
cov[n,m]  = ((fq[n]-qm[n]).(fk[m]-km[m]))/DH
          = (fq[n].fk[m])/DH - qm[n]*km[m]     (exact: 1.fk[m] = DH*km[m])

Both score matrices are low-rank, so with
    A_q[n] = [fq[n]/|fq[n]| ; fq[n]]            # [N, 2*DH]  (cov half RAW)
    A_k[m] = [fk[m]/|fk[m]| ; fk[m]]
    S      = A_k^T @ fv                         # [2*DH, DH] per head (tiny)
    S'     = diag([(1-w)*1 ; (w/DH)*1]) @ S
we get out_h = A_q @ S' + rank-1 correction, where the correction
-(w/DH)*qm (x) colsum(S_cov) is exact because colsum(S_cov) = DH*(km^T fv).
No N x N matrix is ever materialized and no mean-centering pass is needed.

Sharding: rows (B*N = 4096) split across 8 cores (cores 0-3 batch 0,
4-7 batch 1). Two collectives on the single CC stream:
  AR1:  global 8-core AllReduce of qg/kg partials (bf16, 2KB), triggered
        ~12us in; it absorbs the CC entry barrier + first-op wakeup and
        feeds the weight-predictor MLP, which runs during AR-S.
  AR-S: per-batch 4-core-group AllReduce of S partials (bf16, 128KB).
Weights replicated. Output is produced transposed ([D, R] bf16) per core
and fixed up on the host.
"""

import numpy as np

H, DH, B, N, D = 8, 64, 2, 2048, 512
EPS = 1e-5
N_CORES = 8
R = (B * N) // N_CORES  # rows per core = 512
P = 128                 # SBUF partitions

_CACHE = {}


def _build_program():
    import concourse.bacc as bacc
    import concourse.mybir as mybir
    import concourse.tile as tile
    from concourse.masks import make_identity
    from contextlib import ExitStack

    f32 = mybir.dt.float32
    bf16 = mybir.dt.bfloat16
    CDT = bf16
    AX = mybir.AxisListType
    OP = mybir.AluOpType
    AF = mybir.ActivationFunctionType

    nc = bacc.Bacc("TRN2", target_bir_lowering=False, debug=False,
                   num_devices=N_CORES)

    # ---- external I/O (per-core shapes) ----
    q_ext = nc.dram_tensor("q", [R, D], f32, kind="ExternalInput")
    k_ext = nc.dram_tensor("k", [R, D], f32, kind="ExternalInput")
    v_ext = nc.dram_tensor("v", [R, D], f32, kind="ExternalInput")
    winT_ext = nc.dram_tensor("W_inT", [D, D], f32, kind="ExternalInput")    # [d, j]
    woutT_ext = nc.dram_tensor("W_outT", [D, D], f32, kind="ExternalInput")  # [j, dcol]
    lng_ext = nc.dram_tensor("ln_g", [D], f32, kind="ExternalInput")
    lnb_ext = nc.dram_tensor("ln_b", [D], f32, kind="ExternalInput")
    bout_ext = nc.dram_tensor("b_out", [D], f32, kind="ExternalInput")
    w1T_ext = nc.dram_tensor("wp_w1T", [2 * DH, DH], f32, kind="ExternalInput")
    b1_ext = nc.dram_tensor("wp_b1", [DH], f32, kind="ExternalInput")
    wlg_ext = nc.dram_tensor("wp_ln_g", [DH], f32, kind="ExternalInput")
    wlb_ext = nc.dram_tensor("wp_ln_b", [DH], f32, kind="ExternalInput")
    w2_ext = nc.dram_tensor("wp_w2", [DH], f32, kind="ExternalInput")
    b2_ext = nc.dram_tensor("wp_b2", [1], f32, kind="ExternalInput")
    out_ext = nc.dram_tensor("out", [D, R], bf16, kind="ExternalOutput")  # transposed

    NT = R // P  # row tiles per core = 4
    ND = D // P  # feature tiles = 4

    def _patch_pool(pool):
        orig = pool.tile
        def tile_(shape, dtype, tag, **kw):
            return orig(shape, dtype, name=tag, tag=tag, **kw)
        pool.tile = tile_
        return pool

    with tile.TileContext(nc) as tc, ExitStack() as ctx:
        consts = _patch_pool(ctx.enter_context(tc.tile_pool(name="consts", bufs=1)))
        wp = _patch_pool(ctx.enter_context(tc.tile_pool(name="wp", bufs=1)))
        work = _patch_pool(ctx.enter_context(tc.tile_pool(name="work", bufs=4)))
        keep = _patch_pool(ctx.enter_context(tc.tile_pool(name="keep", bufs=1)))
        psum = _patch_pool(ctx.enter_context(tc.tile_pool(name="psum", bufs=1, space="PSUM")))
        pss = _patch_pool(ctx.enter_context(tc.tile_pool(name="pss", bufs=1, space="PSUM")))
        dram = _patch_pool(ctx.enter_context(tc.tile_pool(name="dram", bufs=1, space="DRAM")))

        # ------------- input/weight DMA triggers (critical ones first) -------------
        # sync (HWDGE): k tiles, v tiles, W_out; scalar (HWDGE): q tiles, W_in;
        # gpsimd (SWDGE): small vectors needed early. MLP-only vectors are
        # deferred until after the collectives are queued.
        # Load order matters: ~240 GB/s aggregate means ~20us for all 5.2MB,
        # and each dma_start costs ~2us of serialized ring time — so ONE
        # transfer per tensor. k+q land first (they gate the AR1 trigger),
        # then W_in, then v (gates S/AR-S), then W_out (tail only).
        x_sb = {}
        _ldq = [nc.sync, nc.scalar]
        x_all = {}
        for name, ext, eng in (("k", k_ext, nc.sync), ("q", q_ext, nc.scalar)):
            xa = work.tile([P, NT * D], f32, tag=f"x_{name}", bufs=1)
            eng.dma_start(
                xa[:].rearrange("p (rt d) -> p rt d", rt=NT),
                ext[:].rearrange("(rt p) d -> p rt d", p=P))
            x_all[name] = xa
        wt4 = wp.tile([P, ND * D], f32, tag="wt4")
        nc.scalar.dma_start(
            wt4[:].rearrange("p (di d) -> p di d", di=ND),
            winT_ext[:].rearrange("(di p) d -> p di d", p=P))
        xa = work.tile([P, NT * D], f32, tag="x_v", bufs=1)
        nc.sync.dma_start(
            xa[:].rearrange("p (rt d) -> p rt d", rt=NT),
            v_ext[:].rearrange("(rt p) d -> p rt d", p=P))
        x_all["v"] = xa
        wo4 = wp.tile([P, ND * D], f32, tag="wo4")
        nc.scalar.dma_start(
            wo4[:].rearrange("p (jt d) -> p jt d", jt=ND),
            woutT_ext[:].rearrange("(jt p) d -> p jt d", p=P))
        for name in ("k", "q", "v"):
            for rt in range(NT):
                x_sb[(name, rt)] = x_all[name][:][:, rt * D:(rt + 1) * D]
        g4 = wp.tile([P, ND], f32, tag="g4")
        nc.gpsimd.dma_start(g4[:], lng_ext.rearrange("(di p) -> p di", p=P))
        b4 = wp.tile([P, ND], f32, tag="b4")
        nc.gpsimd.dma_start(b4[:], lnb_ext.rearrange("(di p) -> p di", p=P))
        bout4 = wp.tile([P, ND], f32, tag="bout4")
        nc.gpsimd.dma_start(bout4[:], bout_ext.rearrange("(dt p) -> p dt", p=P))

        # ------------- constants -------------
        # Prime the ACT function tables during the initial DMA window so the
        # 1.3us ACT_TABLE_LOADs don't land mid-pipeline.
        tbl = consts.tile([1, 1], f32, tag="tbl")
        nc.vector.memset(tbl[:], 1.0)
        for fn in (AF.Identity, AF.Square, AF.Sqrt, AF.Sigmoid, AF.Relu):
            nc.scalar.activation(tbl[:], tbl[:], fn)

        ident = consts.tile([P, P], CDT, tag="ident")
        make_identity(nc, ident[:])
        ident8 = consts.tile([H, H], f32, tag="ident8")
        make_identity(nc, ident8[:])
        ones_row = consts.tile([1, P], CDT, tag="ones_row")
        nc.vector.memset(ones_row[:], 1.0)
        negones_col = consts.tile([P, 1], CDT, tag="negones_col")
        nc.vector.memset(negones_col[:], -1.0)
        eps_t = consts.tile([P, 1], f32, tag="eps_t")
        nc.vector.memset(eps_t[:], EPS)
        eighth1 = consts.tile([1, 1], CDT, tag="eighth1")
        nc.vector.memset(eighth1[:], R / float(B * N))
        ind_top = consts.tile([1, P], bf16, tag="ind_top")
        nc.vector.memset(ind_top[:], 0.0)
        nc.vector.memset(ind_top[:, 0:DH], 1.0)
        ind_bot = consts.tile([1, P], bf16, tag="ind_bot")
        nc.vector.memset(ind_bot[:], 0.0)
        nc.vector.memset(ind_bot[:, DH:P], 1.0)

        # ------------- LN + transpose -------------
        xnT = {}

        def layernorm_transpose(name, copy_eng):
            """LN one tensor -> xnT view [P(d), di, rows]."""
            t = keep.tile([P, ND * R], CDT, tag=f"xnT_{name}")
            t4 = t[:].rearrange("p (di r) -> p di r", di=ND)
            for rt in range(NT):
                xt = x_sb[(name, rt)]
                st6 = work.tile([P, 6], f32, tag="st6")
                nc.vector.bn_stats(st6[:], xt)
                mv = work.tile([P, 2], f32, tag="mv")
                nc.vector.bn_aggr(mv[:], st6[:])
                rstd = work.tile([P, 1], f32, tag="rstd")
                nc.scalar.activation(rstd[:], mv[:, 1:2], AF.Sqrt, bias=eps_t[:])
                nc.vector.reciprocal(rstd[:], rstd[:])
                nmr = work.tile([P, 1], f32, tag="nmr")
                nc.vector.scalar_tensor_tensor(
                    nmr[:], mv[:, 0:1], -1.0, rstd[:], op0=OP.mult, op1=OP.mult)
                xn = work.tile([P, D], CDT, tag="xn")
                nc.scalar.activation(xn[:], xt, AF.Identity,
                                     bias=nmr[:], scale=rstd[:])
                tr_ps = psum.tile([P, D], CDT, tag="trps", bufs=3)
                for di in range(ND):
                    nc.tensor.transpose(
                        tr_ps[:, di * P:(di + 1) * P],
                        xn[:, di * P:(di + 1) * P], ident[:])
                if copy_eng == "v":
                    nc.vector.tensor_copy(
                        t4[:, :, rt * P:(rt + 1) * P],
                        tr_ps[:].rearrange("p (di r) -> p di r", di=ND))
                else:
                    nc.scalar.copy(
                        t4[:, :, rt * P:(rt + 1) * P],
                        tr_ps[:].rearrange("p (di r) -> p di r", di=ND))
            xnT[name] = t4
            return t4

        hp = tc.high_priority()
        hp.__enter__()
        layernorm_transpose("k", "v")

        # ------------- weight prep (W_in side) -------------
        wt4v = wt4[:].rearrange("p (di d) -> p di d", di=ND)
        weff = []
        for di in range(ND):
            we = wp.tile([P, D], CDT, tag=f"weff{di}")
            nc.scalar.activation(we[:], wt4v[:, di, :], AF.Identity,
                                 scale=g4[:, di:di + 1])
            weff.append(we)
        bias_ps = pss.tile([1, D], f32, tag="colA")
        for di in range(ND):
            nc.tensor.matmul(bias_ps[:], b4[:, di:di + 1], wt4v[:, di, :],
                             start=(di == 0), stop=(di == ND - 1))
        bias_row = wp.tile([1, D], CDT, tag="bias_row")
        nc.scalar.copy(bias_row[:], bias_ps[:])

        layernorm_transpose("q", "s")
        layernorm_transpose("v", "v")

        # ------------- AR1: global qg/kg partials -------------
        if True:
            xbq = work.tile([P, ND], f32, tag="xbq", bufs=1)
            for di in range(ND):
                nc.vector.reduce_sum(xbq[:, di:di + 1], xnT["q"][:, di, :], axis=AX.X)
            xbk = work.tile([P, ND], f32, tag="xbk", bufs=1)
            for di in range(ND):
                nc.vector.reduce_sum(xbk[:, di:di + 1], xnT["k"][:, di, :], axis=AX.X)
            xbq_bf = work.tile([P, ND], bf16, tag="xbq_bf", bufs=1)
            nc.vector.tensor_scalar_mul(xbq_bf[:], xbq[:], 1.0 / (B * N))
            xbk_bf = work.tile([P, ND], bf16, tag="xbk_bf", bufs=1)
            nc.gpsimd.tensor_scalar_mul(xbk_bf[:], xbk[:], 1.0 / (B * N))
            qg_ps = pss.tile([1, D], f32, tag="colB")
            for di in range(ND):
                nc.tensor.matmul(qg_ps[:], xbq_bf[:, di:di + 1], weff[di][:],
                                 start=(di == 0), stop=False)
            nc.tensor.matmul(qg_ps[:], eighth1[:], bias_row[:], start=False, stop=True)
            kg_ps = pss.tile([1, D], f32, tag="colC")
            for di in range(ND):
                nc.tensor.matmul(kg_ps[:], xbk_bf[:, di:di + 1], weff[di][:],
                                 start=(di == 0), stop=False)
            nc.tensor.matmul(kg_ps[:], eighth1[:], bias_row[:], start=False, stop=True)
            ar1_sb = keep.tile([1, 2 * D], bf16, tag="ar1_sb")
            nc.scalar.copy(ar1_sb[:, 0:D], qg_ps[:])
            nc.vector.tensor_copy(ar1_sb[:, D:2 * D], kg_ps[:])
            ar1_in = dram.tile([2 * D], bf16, tag="ar1_in")
            ar1_out = dram.tile([2 * D], bf16, tag="ar1_out")
            nc.gpsimd.dma_start(ar1_in[:].unsqueeze(0), ar1_sb[:])
            nc.gpsimd.collective_compute(
                "AllReduce", OP.add,
                replica_groups=[list(range(N_CORES))],
                ins=[ar1_in.opt()], outs=[ar1_out.opt()])

        # ------------- projections -------------
        def project_fx(name):
            fx_tiles = []
            for rt in range(NT):
                pj = psum.tile([P, D], f32, tag="projps", bufs=2)
                for di in range(ND):
                    nc.tensor.matmul(
                        pj[:], xnT[name][:, di, rt * P:(rt + 1) * P], weff[di][:],
                        start=(di == 0), stop=False)
                nc.tensor.matmul(pj[:], ones_row[:], bias_row[:],
                                 start=False, stop=True)
                fx = keep.tile([P, D], CDT, tag=f"fx_{name}{rt}")
                nc.scalar.copy(fx[:], pj[:])
                fx_tiles.append(fx)
            return fx_tiles

        def rowstats_and_A(fx_tiles, name, with_mean):
            """Build A [P, (h, 2*DH)]: cos half scaled by 1/|fx_h|, cov half RAW.
            Returns (A_tiles, hmean_tiles)."""
            A_tiles, hm_tiles = [], []
            for rt in range(NT):
                fx = fx_tiles[rt]
                fx3 = fx[:].rearrange("p (h c) -> p h c", h=H)
                sqh = work.tile([P, D], CDT, tag="sqh")
                nc.scalar.activation(sqh[:], fx[:], AF.Square)
                qn2 = work.tile([P, H], f32, tag="qn2")
                nc.vector.reduce_sum(
                    qn2[:], sqh[:].rearrange("p (h c) -> p h c", h=H), axis=AX.X)
                invn = work.tile([P, H], f32, tag="invn")
                nc.scalar.activation(invn[:], qn2[:], AF.Sqrt)
                nc.vector.reciprocal(invn[:], invn[:])
                if with_mean:
                    qsum = work.tile([P, H], f32, tag="qsum")
                    nc.vector.reduce_sum(qsum[:], fx3, axis=AX.X)
                    hm = keep.tile([P, H], CDT, tag=f"hm_{name}{rt}")
                    nc.gpsimd.tensor_scalar_mul(hm[:], qsum[:], 1.0 / DH)
                    hm_tiles.append(hm)
                A = keep.tile([P, 2 * D], CDT, tag=f"A_{name}{rt}")
                A4 = A[:].rearrange("p (h c) -> p h c", h=H)
                eng = nc.vector if (name == "q" or rt < 2) else nc.gpsimd
                eng.tensor_tensor(
                    A4[:, :, 0:DH], fx3,
                    invn[:, :, None].broadcast_to((P, H, DH)), op=OP.mult)
                if rt % 2 == 0:
                    nc.scalar.copy(A4[:, :, DH:2 * DH], fx3)
                else:
                    nc.vector.tensor_copy(A4[:, :, DH:2 * DH], fx3)
                A_tiles.append(A)
            return A_tiles, hm_tiles

        fk_tiles = project_fx("k")
        Ak, _ = rowstats_and_A(fk_tiles, "k", with_mean=False)
        fv_tiles = project_fx("v")

        # ------------- S partials + AR-S (per-batch 4-core groups) -------------
        s_ps = pss.tile([P, H * DH], f32, tag="colA")
        for h in range(H):
            for rt in range(NT):
                nc.tensor.matmul(
                    s_ps[:, h * DH:(h + 1) * DH],
                    Ak[rt][:, h * 2 * DH:(h + 1) * 2 * DH],
                    fv_tiles[rt][:, h * DH:(h + 1) * DH],
                    start=(rt == 0), stop=(rt == NT - 1))
        s_sb = keep.tile([P, H * DH], bf16, tag="s_sb")
        nc.scalar.copy(s_sb[:], s_ps[:])
        SEL = P * H * DH
        ars_in = dram.tile([SEL], bf16, tag="ars_in")
        ars_out = dram.tile([SEL], bf16, tag="ars_out")
        nc.gpsimd.dma_start(ars_in[:].rearrange("(p f) -> p f", p=P), s_sb[:])
        nc.gpsimd.collective_compute(
            "AllReduce", OP.add,
            replica_groups=[[0, 1, 2, 3], [4, 5, 6, 7]],
            ins=[ars_in.opt()], outs=[ars_out.opt()])
        hp.__exit__(None, None, None)

        # ------------- q side (overlaps AR-S) -------------
        fq_tiles = project_fx("q")
        Aq, hmq = rowstats_and_A(fq_tiles, "q", with_mean=True)
        AqT = []
        for h in range(H):
            aq_ps = psum.tile([P, R], CDT, tag="trps", bufs=3)
            for rt in range(NT):
                nc.tensor.transpose(
                    aq_ps[:, rt * P:(rt + 1) * P],
                    Aq[rt][:, h * 2 * DH:(h + 1) * 2 * DH], ident[:])
            at = keep.tile([P, R], CDT, tag=f"AqT{h}")
            if h % 2 == 0:
                nc.vector.tensor_copy(at[:], aq_ps[:])
            else:
                nc.scalar.copy(at[:], aq_ps[:])
            AqT.append(at)
        # qmT [h, rows]: per-head row-means of fq, transposed
        qm_ps = pss.tile([H, R], CDT, tag="colB")
        for rt in range(NT):
            nc.tensor.transpose(
                qm_ps[:, rt * P:(rt + 1) * P], hmq[rt][:], ident[:])
        qmT = keep.tile([H, R], bf16, tag="qmT")
        nc.vector.tensor_copy(qmT[:], qm_ps[:])
        # flatten to one partition (via DRAM) so the tail's rank-1 matmuls
        # get base-0 APs
        qm_dram = dram.tile([H * R], bf16, tag="qm_dram")
        nc.gpsimd.dma_start(qm_dram[:].rearrange("(h r) -> h r", h=H), qmT[:])
        qm_flat = keep.tile([1, H * R], bf16, tag="qm_flat")
        nc.gpsimd.dma_start(qm_flat[:], qm_dram[:].unsqueeze(0))

        # ------------- deferred weights (needed only post-AR1) -------------
        w1T = wp.tile([2 * DH, DH], f32, tag="w1T")
        nc.gpsimd.dma_start(w1T[:], w1T_ext[:])
        b1_rep = wp.tile([H, DH], f32, tag="b1_rep")
        nc.gpsimd.dma_start(b1_rep[:], b1_ext[None, :].to_broadcast((H, DH)))
        wlg_rep = wp.tile([H, DH], f32, tag="wlg_rep")
        nc.gpsimd.dma_start(wlg_rep[:], wlg_ext[None, :].to_broadcast((H, DH)))
        wlb_rep = wp.tile([H, DH], f32, tag="wlb_rep")
        nc.gpsimd.dma_start(wlb_rep[:], wlb_ext[None, :].to_broadcast((H, DH)))
        w2_rep = wp.tile([H, DH], f32, tag="w2_rep")
        nc.gpsimd.dma_start(w2_rep[:], w2_ext[None, :].to_broadcast((H, DH)))
        b2_col = wp.tile([H, 1], f32, tag="b2_col")
        nc.gpsimd.dma_start(b2_col[:], b2_ext[None, :].to_broadcast((H, 1)))
        w1T_bf = wp.tile([2 * DH, DH], bf16, tag="w1T_bf")
        nc.vector.tensor_copy(w1T_bf[:], w1T[:])
        wo4v = wo4[:].rearrange("p (jt d) -> p jt d", jt=ND)
        woutT = []
        for jt in range(ND):
            wo = wp.tile([P, D], CDT, tag=f"woutT{jt}")
            nc.scalar.copy(wo[:], wo4v[:, jt, :])
            woutT.append(wo)

        # ------------- weight-predictor MLP (after AR1, during AR-S) -------------
        featT = keep.tile([2 * DH, H], bf16, tag="featT")
        nc.sync.dma_start(
            featT[0:DH, :], ar1_out[0:D].rearrange("(h c) -> c h", h=H))
        nc.scalar.dma_start(
            featT[DH:2 * DH, :], ar1_out[D:2 * D].rearrange("(h c) -> c h", h=H))
        hid_ps = pss.tile([H, DH], f32, tag="colC")
        nc.tensor.matmul(hid_ps[:], featT[:], w1T_bf[:], start=True, stop=True)
        hid = keep.tile([H, DH], f32, tag="hid")
        nc.vector.tensor_tensor(hid[:], hid_ps[:], b1_rep[:], op=OP.add)
        hst6 = keep.tile([H, 6], f32, tag="hst6")
        nc.vector.bn_stats(hst6[:], hid[:])
        hmv = keep.tile([H, 2], f32, tag="hmv")
        nc.vector.bn_aggr(hmv[:], hst6[:])
        hrstd = keep.tile([H, 1], f32, tag="hrstd")
        nc.scalar.activation(hrstd[:], hmv[:, 1:2], AF.Sqrt, bias=eps_t[0:H, :])
        nc.vector.reciprocal(hrstd[:], hrstd[:])
        hnmr = keep.tile([H, 1], f32, tag="hnmr")
        nc.vector.scalar_tensor_tensor(
            hnmr[:], hmv[:, 0:1], -1.0, hrstd[:], op0=OP.mult, op1=OP.mult)
        hln = keep.tile([H, DH], f32, tag="hln")
        nc.scalar.activation(hln[:], hid[:], AF.Identity,
                             bias=hnmr[:], scale=hrstd[:])
        nc.vector.tensor_tensor(hln[:], hln[:], wlg_rep[:], op=OP.mult)
        nc.vector.tensor_tensor(hln[:], hln[:], wlb_rep[:], op=OP.add)
        nc.scalar.activation(hln[:], hln[:], AF.Relu)
        lscr = keep.tile([H, DH], f32, tag="lscr")
        nc.vector.tensor_tensor(lscr[:], hln[:], w2_rep[:], op=OP.mult)
        logit = keep.tile([H, 1], f32, tag="logit")
        nc.vector.reduce_sum(logit[:], lscr[:], axis=AX.X)
        nc.vector.tensor_tensor(logit[:], logit[:], b2_col[:], op=OP.add)
        wcol = keep.tile([H, 1], f32, tag="wcol")
        nc.scalar.activation(wcol[:], logit[:], AF.Sigmoid)
        wr_ps = pss.tile([1, H], f32, tag="colC")
        nc.tensor.transpose(wr_ps[:], wcol[:], ident8[:])
        wrow = keep.tile([1, H], f32, tag="wrow")
        nc.vector.tensor_copy(wrow[:], wr_ps[:])
        omw = keep.tile([1, H], bf16, tag="omw")
        nc.vector.tensor_scalar(omw[:], wrow[:], scalar1=-1.0, scalar2=1.0,
                                op0=OP.mult, op1=OP.add)
        wdh = keep.tile([1, H], bf16, tag="wdh")
        nc.vector.tensor_scalar_mul(wdh[:], wrow[:], 1.0 / DH)
        wsc_ps = pss.tile([P, H], f32, tag="colB")
        nc.tensor.matmul(wsc_ps[:], ind_top[:], omw[:], start=True, stop=False)
        nc.tensor.matmul(wsc_ps[:], ind_bot[:], wdh[:], start=False, stop=True)
        wsc = keep.tile([P, H], bf16, tag="wsc")
        nc.vector.tensor_copy(wsc[:], wsc_ps[:])

        # ------------- tail: blend-scale + rank-1 fix + output projection -------
        s0 = keep.tile([P, H * DH], bf16, tag="s0")
        nc.sync.dma_start(s0[:], ars_out[:].rearrange("(p f) -> p f", p=P))
        s_sc = keep.tile([P, H * DH], CDT, tag="s_sc")
        nc.vector.tensor_tensor(
            s_sc[:].rearrange("p (h c) -> p h c", h=H),
            s0[:].rearrange("p (h c) -> p h c", h=H),
            wsc[:, :, None].broadcast_to((P, H, DH)), op=OP.mult)
        # t[c] = -(w/DH) * colsum(S_cov)[c]  (cov rows of s_sc are w/DH-scaled)
        t_ps = pss.tile([1, H * DH], f32, tag="colC")
        nc.tensor.matmul(t_ps[:], negones_col[DH:P, :], s_sc[DH:P, :],
                         start=True, stop=True)
        t_row = keep.tile([1, H * DH], bf16, tag="t_row")
        nc.vector.tensor_copy(t_row[:], t_ps[:])

        foutT = []
        for jt in range(ND):
            ft = keep.tile([P, R], CDT, tag=f"foutT{jt}")
            foutT.append(ft)

        def store_dt(o_ps_tile, dt_):
            o_sb = work.tile([P, R], bf16, tag="o_sb")
            if dt_ % 2 == 0:
                nc.scalar.activation(o_sb[:], o_ps_tile[:], AF.Identity,
                                     bias=bout4[:, dt_:dt_ + 1], scale=1.0)
            else:
                nc.vector.tensor_scalar_add(o_sb[:], o_ps_tile[:], bout4[:, dt_:dt_ + 1])
            _ldq[dt_ % 2].dma_start(out_ext[dt_ * P:(dt_ + 1) * P, :], o_sb[:])

        # 4 concurrent output accumulators: 2 projps bufs + the colA/colB
        # banks (their earlier occupants are dead by now).
        o_ps = [psum.tile([P, R], f32, tag="projps", bufs=2) for _ in range(2)]
        o_ps.append(pss.tile([P, R], f32, tag="colA"))
        o_ps.append(pss.tile([P, R], f32, tag="colB"))
        for jt in range(ND):
            for hh in range(2):
                h = 2 * jt + hh
                m_ps = psum.tile([DH, R], f32, tag="trps", bufs=3)
                nc.tensor.matmul(m_ps[:], s_sc[:, h * DH:(h + 1) * DH], AqT[h][:],
                                 start=True, stop=False)
                nc.tensor.matmul(m_ps[:], t_row[0:1, h * DH:(h + 1) * DH],
                                 qm_flat[0:1, h * R:(h + 1) * R],
                                 start=False, stop=True)
                dst = foutT[jt][hh * DH:(hh + 1) * DH, :]
                if hh == 0:
                    nc.scalar.copy(dst, m_ps[:])
                else:
                    nc.vector.tensor_copy(dst, m_ps[:])
            for dt_ in range(ND):
                nc.tensor.matmul(
                    o_ps[dt_][:], woutT[jt][:, dt_ * P:(dt_ + 1) * P], foutT[jt][:],
                    start=(jt == 0), stop=(jt == ND - 1))
        for dt_ in range(ND):
            store_dt(o_ps[dt_], dt_)

    nc.finalize()
    return nc


def _get_program():
    if "nc" not in _CACHE:
        _CACHE["nc"] = _build_program()
    return _CACHE["nc"]


def _make_in_maps(inputs):
    q = np.ascontiguousarray(np.asarray(inputs["q"], np.float32).reshape(B * N, D))
    k = np.ascontiguousarray(np.asarray(inputs["k"], np.float32).reshape(B * N, D))
    v = np.ascontiguousarray(np.asarray(inputs["v"], np.float32).reshape(B * N, D))
    shared = {
        "W_inT": np.ascontiguousarray(np.asarray(inputs["W_in"], np.float32).T),
        "W_outT": np.ascontiguousarray(np.asarray(inputs["W_out"], np.float32).T),
        "ln_g": np.asarray(inputs["ln_g"], np.float32),
        "ln_b": np.asarray(inputs["ln_b"], np.float32),
        "b_out": np.asarray(inputs["b_out"], np.float32),
        "wp_w1T": np.ascontiguousarray(np.asarray(inputs["wp_w1"], np.float32).T),
        "wp_b1": np.asarray(inputs["wp_b1"], np.float32),
        "wp_ln_g": np.asarray(inputs["wp_ln_g"], np.float32),
        "wp_ln_b": np.asarray(inputs["wp_ln_b"], np.float32),
        "wp_w2": np.ascontiguousarray(np.asarray(inputs["wp_w2"], np.float32).reshape(DH)),
        "wp_b2": np.asarray(inputs["wp_b2"], np.float32).reshape(1),
    }
    in_maps = []
    for c in range(N_CORES):
        m = dict(shared)
        sl = slice(c * R, (c + 1) * R)
        m["q"] = np.ascontiguousarray(q[sl])
        m["k"] = np.ascontiguousarray(k[sl])
        m["v"] = np.ascontiguousarray(v[sl])
        in_maps.append(m)
    return in_maps


def _gather(results):
    out = np.empty((B * N, D), np.float32)
    for c in range(N_CORES):
        out[c * R:(c + 1) * R, :] = np.asarray(results[c]["out"], np.float32).T
    return out.reshape(B, N, D)


def _run(inputs, trace=False, trace_cores=None):
    from concourse.bass_utils import run_bass_kernel_spmd
    nc = _get_program()
    in_maps = _make_in_maps(inputs)
    res = run_bass_kernel_spmd(
        nc, in_maps, core_ids=list(range(N_CORES)),
        trace=trace, trace_cores=trace_cores)
    return _gather(res.results), res


def kernel(**inputs) -> np.ndarray:
    out, _ = _run(inputs, trace=False)
    return out


def run_traced(inputs, trace_cores=None):
    return _run(inputs, trace=True, trace_cores=trace_cores)
